# revision 38
# baseline (speedup 1.0000x reference)
"""Euclidean distance layer on 8 Trainium2 NeuronCores.

out[b, o] = || x[b, :] - weight[:, o] ||_2
x: [512, 256] f32, weight: [256, 1024] f32 -> out: [512, 1024] f32

Sharding: tensor-parallel over output features (8 x 128 columns per core).

Transposed-psum fp8 design: psum[o_local, b] with k=256 contraction:

  ps  [o, b] = sum_k  w[k,o] * x[k,b]        (1 DR mm, lhsT=w)
             + sum_k (-0.5) * xsq[k,b]       (1 DR mm, lhsT=const -0.5)
  ps_w[o, 0] = sum_k  wsq[k,o] * 1           (1 DR mm, n=1)
  out [o, b] = sqrt(-2*ps + bias=wcol)       (1 ACT, f16 out)

Inputs fp8 e4m3, contiguous per partition, as TWO blob DMAs on the
sync queue: blob A = [x-chunk0 | w] (768B rows), blob B = x-chunk1
(inter-DMA queue gaps and semaphore-trickle tails cost ~0.5us per extra
DMA, and the gpsimd/scalar queues are slow or force act-table reloads,
so the one fast queue carries two merged transfers; every matmul
explicitly waits on the DMA sems of all tensors it reads). The three
sball regions [xc0 | w | xc1] are addressed by strided APs so the
DoubleRow matmuls see [p, 2, free] views.
Squares are split: scalar-ACT takes b-half A of each x-chunk, DVE takes
b-half B plus the w-square (slotted in the w-arrival gap) and the
||w||^2 psum->SBUF copy. Square/Sqrt share act table set 3; the dummy
sqrt is scalar's first instruction so exactly one 1.28us table load
lands at block start, hidden under the input DMA. PE overlaps the main
DR matmul with the square wave. Output DMA issues from scalar with no
completion wait; the fixed ~7.8us NEFF epilogue (full semaphore-file
reset) outlasts the transfer. Host work: layout/dtype prep + T/concat.
"""

from contextlib import ExitStack

import numpy as np

B = 512      # batch
WOFF = 512 + 256  # chunk1 column offset inside sball
BH = 224     # scalar's share of each chunk's squares (DVE is faster)
K = 256      # inputSize (contraction dim)
NOUT = 1024  # outputSize
NCORES = 8
NLOC = NOUT // NCORES  # 128 output features per core
P = 128                # partitions
KT = K // P            # 2 contraction chunks

_NC = None  # cached compiled Bass program (same SPMD program on all cores)


def _build():
    import concourse.bass as bass
    from concourse import bacc, mybir

    f32 = mybir.dt.float32
    f16 = mybir.dt.float16
    f8 = mybir.dt.float8e4
    DR = mybir.MatmulPerfMode.DoubleRow
    Sqrt = mybir.ActivationFunctionType.Sqrt
    Square = mybir.ActivationFunctionType.Square

    nc = bacc.Bacc(
        "TRN2", target_bir_lowering=False, debug=False, num_devices=NCORES
    )

    blob_a = nc.dram_tensor("blob_a", [P, B + KT * NLOC], f8,
                            kind="ExternalInput")  # [xc0 | w]
    blob_b = nc.dram_tensor("blob_b", [P, B], f8, kind="ExternalInput")
    out = nc.dram_tensor("out", [P, B], f16, kind="ExternalOutput")

    with ExitStack() as ctx:
        e = ctx.enter_context
        sball = e(nc.sbuf_tensor("sball", [P, B + KT * NLOC + B], f8))
        xsq = e(nc.sbuf_tensor("xsq", [P, KT, B], f8))
        wlsq = e(nc.sbuf_tensor("wlsq", [P, KT, NLOC], f8))
        neghalf = e(nc.sbuf_tensor("neghalf", [P, KT, NLOC], f8))
        ones1 = e(nc.sbuf_tensor("ones1", [P, KT, 1], f8))
        wcol = e(nc.sbuf_tensor("wcol", [P, 1], f32))
        out_sb = e(nc.sbuf_tensor("out_sb", [P, B], f16))
        dumm = e(nc.sbuf_tensor("dumm", [1, 1], f32))

        ps = e(nc.psum_tensor("ps", [P, B], f32))       # one full bank
        ps_w = e(nc.psum_tensor("ps_w", [P, 1], f32))   # ||w||^2 column

        s_a = e(nc.semaphore("s_a"))  # blob A: x chunk0 + w
        s_b = e(nc.semaphore("s_b"))  # blob B: x chunk1
        s_wsq = e(nc.semaphore("s_wsq"))
        s_sq = e(nc.semaphore("s_sq"))      # 4 = all quarter-squares done
        s_mm = e(nc.semaphore("s_mm"))      # 1 = ps_w, 2 = ps done
        s_wcol = e(nc.semaphore("s_wcol"))
        s_sqrt = e(nc.semaphore("s_sqrt"))
        s_out = e(nc.semaphore("s_out"))    # inc only; no waiter
        s_dum = e(nc.semaphore("s_dum"))
        s_cst = e(nc.semaphore("s_cst"))

        WOFF_L = B + KT * NLOC  # chunk1 offset in sball
        # [p, c, o] view of w at offset B; [p, c, b] view of x with c-stride WOFF_L
        wview = bass.AP(
            tensor=sball, offset=B,
            ap=[[sball.shape[1], P], [NLOC, KT], [1, NLOC]],
        )
        xview = bass.AP(
            tensor=sball, offset=0,
            ap=[[sball.shape[1], P], [WOFF_L, KT], [1, B]],
        )

        block = e(nc.Block())

        @block.sync
        def _(sync):
            sync.dma_start(
                out=sball[:, 0 : B + KT * NLOC], in_=blob_a[:, :]
            ).then_inc(s_a, 16)
            sync.dma_start(
                out=sball[:, B + KT * NLOC :], in_=blob_b[:, :]
            ).then_inc(s_b, 16)


        @block.scalar
        def _(scalar):
            # dummy sqrt FIRST: exactly one act-table load (set 3 covers
            # Sqrt and Square), hoisted to block start
            scalar.wait_ge(s_dum, 1)
            scalar.activation(dumm[:, :], dumm[:, :], Sqrt)
            # square b-half A of each chunk as it lands (DVE takes half B)
            scalar.wait_ge(s_a, 16)
            scalar.activation(
                xsq[:, 0, 0:BH], sball[:, 0:BH], Square
            ).then_inc(s_sq)
            scalar.wait_ge(s_b, 16)
            scalar.activation(
                xsq[:, 1, 0:BH], sball[:, WOFF + 0 : WOFF + BH], Square
            ).then_inc(s_sq)
            scalar.wait_ge(s_mm, 2)
            scalar.wait_ge(s_wcol, 1)
            scalar.activation(
                out_sb[:, :], ps[:, :], Sqrt, bias=wcol[:, :], scale=-2.0
            ).then_inc(s_sqrt)
            scalar.wait_ge(s_sqrt, 1)
            scalar.dma_start(
                out=out[:, :], in_=out_sb[:, :]
            ).then_inc(s_out, 16)
            # no completion wait: the fixed NEFF epilogue outlasts the
            # transfer; nrt reads outputs only after full teardown.

        @block.vector
        def _(vector):
            vector.memset(dumm[:, :], 1.0).then_inc(s_dum)
            vector.memset(neghalf[:, :, :], -0.5)
            vector.memset(ones1[:, :, :], 1.0).then_inc(s_cst)
            vector.wait_ge(s_a, 16)
            vector.tensor_mul(
                xsq[:, 0, BH:B], sball[:, BH:B], sball[:, BH:B]
            ).then_inc(s_sq)
            vector.tensor_mul(
                wlsq[:, :, :], wview, wview
            ).then_inc(s_wsq)
            vector.wait_ge(s_b, 16)
            vector.tensor_mul(
                xsq[:, 1, BH:B], sball[:, WOFF + BH : WOFF + B],
                sball[:, WOFF + BH : WOFF + B]
            ).then_inc(s_sq)
            vector.wait_ge(s_mm, 2)
            vector.tensor_copy(wcol[:, :], ps_w[:, :]).then_inc(s_wcol)

        @block.tensor
        def _(tensor):
            # main x.w (DR, k=256 in one shot; x chunks strided in sball)
            tensor.wait_ge(s_a, 16)
            tensor.wait_ge(s_b, 16)
            tensor.matmul(
                ps[:, :], lhsT=wview, rhs=xview,
                start=True, stop=False, perf_mode=DR, skip_group_check=True,
            )
            # -0.5*||x||^2 (DR, both chunks, full width) directly after
            # main so its issue rides the PE pipeline slot
            tensor.wait_ge(s_sq, 4)
            tensor.matmul(
                ps[:, :], lhsT=neghalf[:, :, :], rhs=xsq[:, :, :],
                start=False, stop=True, perf_mode=DR, skip_group_check=True,
            ).then_inc(s_mm)  # = 1 (ps done)
            # ||w||^2 column after; its bias chain still beats the sqrt
            tensor.wait_ge(s_cst, 1)
            tensor.wait_ge(s_wsq, 1)
            tensor.matmul(
                ps_w[:, :], lhsT=wlsq[:, :, :], rhs=ones1[:, :, :],
                start=True, stop=True, perf_mode=DR, skip_group_check=True,
            ).then_inc(s_mm)  # = 2 (ps_w done)

    nc.compile()
    return nc


def _get_nc():
    global _NC
    if _NC is None:
        _NC = _build()
    return _NC


def _np_f8():
    from concourse import mybir

    return mybir.dt.np(mybir.dt.float8e4)


def _make_in_maps(x: np.ndarray, weight: np.ndarray):
    f8 = _np_f8()
    xf = x.astype(f8)
    wf = weight.astype(f8)
    # xh[p, c, b] = x[b, c*128+p]
    xh = xf.T.reshape(KT, P, B)
    blob_b = np.ascontiguousarray(xh[1])
    maps = []
    for c in range(NCORES):
        wl = wf[:, c * NLOC : (c + 1) * NLOC]  # [256, 128]
        whc = wl.reshape(KT, P, NLOC).transpose(1, 0, 2).reshape(P, KT * NLOC)
        blob_a = np.ascontiguousarray(np.concatenate([xh[0], whc], axis=1))
        maps.append({"blob_a": blob_a, "blob_b": blob_b})
    return maps


def run(x: np.ndarray, weight: np.ndarray, trace: bool = False):
    """Returns (full_output, BassKernelResults)."""
    from concourse.bass_utils import run_bass_kernel_spmd

    nc = _get_nc()
    res = run_bass_kernel_spmd(
        nc, _make_in_maps(x, weight), core_ids=list(range(NCORES)), trace=trace
    )
    # out[o_local, b] per core -> full [B, NOUT] f32
    full = np.concatenate(
        [res.results[c]["out"].T.astype(np.float32) for c in range(NCORES)],
        axis=1,
    )
    return full, res


def kernel(x: np.ndarray, weight: np.ndarray) -> np.ndarray:
    return run(x, weight)[0]


# revision 39
# speedup vs baseline: 1.0026x; 1.0026x over previous
"""Euclidean distance layer on 8 Trainium2 NeuronCores.

out[b, o] = || x[b, :] - weight[:, o] ||_2
x: [512, 256] f32, weight: [256, 1024] f32 -> out: [512, 1024] f32

Sharding: tensor-parallel over output features (8 x 128 columns per core).

Transposed-psum fp8 design: psum[o_local, b] with k=256 contraction:

  ps  [o, b] = sum_k  w[k,o] * x[k,b]        (1 DR mm, lhsT=w)
             + sum_k (-0.5) * xsq[k,b]       (1 DR mm, lhsT=const -0.5)
  ps_w[o, 0] = sum_k  wsq[k,o] * 1           (1 DR mm, n=1)
  out [o, b] = sqrt(-2*ps + bias=wcol)       (1 ACT, f16 out)

Inputs fp8 e4m3, contiguous per partition, as TWO blob DMAs on the
sync queue: blob A = [x-chunk0 | w] (768B rows), blob B = x-chunk1
(inter-DMA queue gaps and semaphore-trickle tails cost ~0.5us per extra
DMA, and the gpsimd/scalar queues are slow or force act-table reloads,
so the one fast queue carries two merged transfers; every matmul
explicitly waits on the DMA sems of all tensors it reads). The three
sball regions [xc0 | w | xc1] are addressed by strided APs so the
DoubleRow matmuls see [p, 2, free] views.
Squares are split: scalar-ACT takes b-half A of each x-chunk, DVE takes
b-half B plus the w-square (slotted in the w-arrival gap) and the
||w||^2 psum->SBUF copy. Square/Sqrt share act table set 3; the dummy
sqrt is scalar's first instruction so exactly one 1.28us table load
lands at block start, hidden under the input DMA. PE overlaps the main
DR matmul with the square wave. Output DMA issues from scalar with no
completion wait; the fixed ~7.8us NEFF epilogue (full semaphore-file
reset) outlasts the transfer. Host work: layout/dtype prep + T/concat.
"""

from contextlib import ExitStack

import numpy as np

B = 512      # batch
WOFF = 512 + 256  # chunk1 column offset inside sball
BH = 224     # scalar's share of each chunk's squares (DVE is faster)
K = 256      # inputSize (contraction dim)
NOUT = 1024  # outputSize
NCORES = 8
NLOC = NOUT // NCORES  # 128 output features per core
P = 128                # partitions
KT = K // P            # 2 contraction chunks

_NC = None  # cached compiled Bass program (same SPMD program on all cores)


def _build():
    import concourse.bass as bass
    from concourse import bacc, mybir

    f32 = mybir.dt.float32
    f16 = mybir.dt.float16
    f8 = mybir.dt.float8e4
    DR = mybir.MatmulPerfMode.DoubleRow
    Sqrt = mybir.ActivationFunctionType.Sqrt
    Square = mybir.ActivationFunctionType.Square

    nc = bacc.Bacc(
        "TRN2", target_bir_lowering=False, debug=False, num_devices=NCORES
    )

    blob_a = nc.dram_tensor("blob_a", [P, B + KT * NLOC], f8,
                            kind="ExternalInput")  # [xc0 | w]
    blob_b = nc.dram_tensor("blob_b", [P, B], f8, kind="ExternalInput")
    out = nc.dram_tensor("out", [P, B], f16, kind="ExternalOutput")

    with ExitStack() as ctx:
        e = ctx.enter_context
        sball = e(nc.sbuf_tensor("sball", [P, B + KT * NLOC + B], f8))
        xsq = e(nc.sbuf_tensor("xsq", [P, KT, B], f8))
        wlsq = e(nc.sbuf_tensor("wlsq", [P, KT, NLOC], f8))
        neghalf = e(nc.sbuf_tensor("neghalf", [P, KT, NLOC], f8))
        ones1 = e(nc.sbuf_tensor("ones1", [P, KT, 1], f8))
        wcol = e(nc.sbuf_tensor("wcol", [P, 1], f32))
        out_sb = e(nc.sbuf_tensor("out_sb", [P, B], f16))
        dumm = e(nc.sbuf_tensor("dumm", [1, 1], f32))

        ps = e(nc.psum_tensor("ps", [P, B], f32))       # one full bank
        ps_w = e(nc.psum_tensor("ps_w", [P, 1], f32))   # ||w||^2 column

        s_a = e(nc.semaphore("s_a"))  # blob A: x chunk0 + w
        s_b = e(nc.semaphore("s_b"))  # blob B: x chunk1
        s_wsq = e(nc.semaphore("s_wsq"))
        s_sq = e(nc.semaphore("s_sq"))      # 4 = all quarter-squares done
        s_mm = e(nc.semaphore("s_mm"))      # 1 = ps_w, 2 = ps done
        s_wcol = e(nc.semaphore("s_wcol"))
        s_sqrt = e(nc.semaphore("s_sqrt"))
        s_out = e(nc.semaphore("s_out"))    # inc only; no waiter
        s_dum = e(nc.semaphore("s_dum"))
        s_cst = e(nc.semaphore("s_cst"))

        WOFF_L = B + KT * NLOC  # chunk1 offset in sball
        # [p, c, o] view of w at offset B; [p, c, b] view of x with c-stride WOFF_L
        wview = bass.AP(
            tensor=sball, offset=B,
            ap=[[sball.shape[1], P], [NLOC, KT], [1, NLOC]],
        )
        xview = bass.AP(
            tensor=sball, offset=0,
            ap=[[sball.shape[1], P], [WOFF_L, KT], [1, B]],
        )

        block = e(nc.Block())

        @block.sync
        def _(sync):
            sync.dma_start(
                out=sball[:, 0 : B + KT * NLOC], in_=blob_a[:, :]
            ).then_inc(s_a, 16)
            sync.dma_start(
                out=sball[:, B + KT * NLOC :], in_=blob_b[:, :]
            ).then_inc(s_b, 16)


        @block.scalar
        def _(scalar):
            # dummy sqrt FIRST: exactly one act-table load (set 3 covers
            # Sqrt and Square), hoisted to block start
            scalar.wait_ge(s_dum, 1)
            scalar.activation(dumm[:, :], dumm[:, :], Sqrt)
            # square b-half A of each chunk as it lands (DVE takes half B)
            scalar.wait_ge(s_a, 16)
            scalar.activation(
                xsq[:, 0, 0:BH], sball[:, 0:BH], Square
            ).then_inc(s_sq)
            scalar.wait_ge(s_b, 16)
            scalar.activation(
                xsq[:, 1, 0:BH], sball[:, WOFF + 0 : WOFF + BH], Square
            ).then_inc(s_sq)
            scalar.wait_ge(s_mm, 2)
            scalar.wait_ge(s_wcol, 1)
            scalar.activation(
                out_sb[:, :], ps[:, :], Sqrt, bias=wcol[:, :], scale=-2.0
            ).then_inc(s_sqrt)
            scalar.wait_ge(s_sqrt, 1)
            scalar.dma_start(
                out=out[:, :], in_=out_sb[:, :]
            ).then_inc(s_out, 16)
            # no completion wait: the fixed NEFF epilogue outlasts the
            # transfer; nrt reads outputs only after full teardown.

        @block.vector
        def _(vector):
            vector.memset(dumm[:, :], 1.0).then_inc(s_dum)
            vector.memset(neghalf[:, :, :], -0.5)
            vector.memset(ones1[:, :, :], 1.0).then_inc(s_cst)
            vector.wait_ge(s_a, 16)
            vector.tensor_mul(
                xsq[:, 0, BH:B], sball[:, BH:B], sball[:, BH:B]
            ).then_inc(s_sq)
            vector.tensor_mul(
                wlsq[:, :, :], wview, wview
            ).then_inc(s_wsq)
            vector.wait_ge(s_b, 16)
            vector.tensor_mul(
                xsq[:, 1, BH:B], sball[:, WOFF + BH : WOFF + B],
                sball[:, WOFF + BH : WOFF + B]
            ).then_inc(s_sq)
            vector.wait_ge(s_mm, 1)
            vector.tensor_copy(wcol[:, :], ps_w[:, :]).then_inc(s_wcol)

        @block.tensor
        def _(tensor):
            # main x.w (DR, k=256 in one shot; x chunks strided in sball)
            tensor.wait_ge(s_a, 16)
            tensor.wait_ge(s_b, 16)
            tensor.matmul(
                ps[:, :], lhsT=wview, rhs=xview,
                start=True, stop=False, perf_mode=DR, skip_group_check=True,
            )
            # ||w||^2 column (n=1, slots into the main stream's shadow)
            tensor.wait_ge(s_cst, 1)
            tensor.wait_ge(s_wsq, 1)
            tensor.matmul(
                ps_w[:, :], lhsT=wlsq[:, :, :], rhs=ones1[:, :, :],
                start=True, stop=True, perf_mode=DR, skip_group_check=True,
            ).then_inc(s_mm)  # = 1
            # -0.5*||x||^2 (DR, both chunks, full width)
            tensor.wait_ge(s_sq, 4)
            tensor.matmul(
                ps[:, :], lhsT=neghalf[:, :, :], rhs=xsq[:, :, :],
                start=False, stop=True, perf_mode=DR, skip_group_check=True,
            ).then_inc(s_mm)  # = 2

    nc.compile()
    return nc


def _get_nc():
    global _NC
    if _NC is None:
        _NC = _build()
    return _NC


def _np_f8():
    from concourse import mybir

    return mybir.dt.np(mybir.dt.float8e4)


def _make_in_maps(x: np.ndarray, weight: np.ndarray):
    f8 = _np_f8()
    xf = x.astype(f8)
    wf = weight.astype(f8)
    # xh[p, c, b] = x[b, c*128+p]
    xh = xf.T.reshape(KT, P, B)
    blob_b = np.ascontiguousarray(xh[1])
    maps = []
    for c in range(NCORES):
        wl = wf[:, c * NLOC : (c + 1) * NLOC]  # [256, 128]
        whc = wl.reshape(KT, P, NLOC).transpose(1, 0, 2).reshape(P, KT * NLOC)
        blob_a = np.ascontiguousarray(np.concatenate([xh[0], whc], axis=1))
        maps.append({"blob_a": blob_a, "blob_b": blob_b})
    return maps


def run(x: np.ndarray, weight: np.ndarray, trace: bool = False):
    """Returns (full_output, BassKernelResults)."""
    from concourse.bass_utils import run_bass_kernel_spmd

    nc = _get_nc()
    res = run_bass_kernel_spmd(
        nc, _make_in_maps(x, weight), core_ids=list(range(NCORES)), trace=trace
    )
    # out[o_local, b] per core -> full [B, NOUT] f32
    full = np.concatenate(
        [res.results[c]["out"].T.astype(np.float32) for c in range(NCORES)],
        axis=1,
    )
    return full, res


def kernel(x: np.ndarray, weight: np.ndarray) -> np.ndarray:
    return run(x, weight)[0]
